# revision 9
# baseline (speedup 1.0000x reference)
"""Multi-head attention (B=4, S=2048, D=1024, H=16) on 8 Trainium2 cores.

Sharding: DP=4 over batch x TP=2 over heads (8 heads/core).

Single fused pipeline per core, built so the tensor engine never idles:
  - inputs are HOST-PACKED chunk-major ([128, sc, d, s'] with row index
    d*128+p) so every input DMA is 128 x 8KB contiguous descriptors
  - attention runs as 16 pairs (qp quarter x hp head-pair); ALL projection
    work (K/V/Q/O) is emitted as "filler" matmul chains inside the pairs'
    kc slots, sized so per-slot tensor time (~1.1-1.3us) stays above the
    scalar engine's exp time (~1.1us) -- the scalar engine never gates.
  - scores S^T = kT.T @ qq per 128-key chunk; P = exp(S/8) mostly on the
    scalar engine's exp LUT; per pair, kc=0,1 run on the vector engine via
    a custom 2-op chain (deg-4 Taylor poly of e^(x/128), then ^16) and
    their PV accumulation is rotated to the END of the pair so the DVE
    FIFO latency is hidden (PV order = [2..15, 0, 1]).
  - O^T = [v | 1].T @ P (ones column = softmax denominator in row 64).
  - normalize WITHOUT a DRAM round-trip: denominator row is evicted to
    bf16, broadcast to 64 partitions by a K=1 matmul (ones lhsT), then
    reciprocal_approx_fast + multiply on the vector engine. The hloc=1
    half is partition-shifted into AT[64:128] by an SBUF->SBUF DMA.
  - output projection Y_g = A_g @ Wo_g.T chains are fillers too; Y bf16.
Host sums the two TP partials per batch and adds bo + Wo @ bv (the v-bias
commutes through the normalized softmax).
"""

import os
import sys

sys.path.insert(0, "/opt/trn_rl_repo")
os.environ.setdefault("MYCRO_LOCAL_CACHE", "1")

import numpy as np
import ml_dtypes
import concourse.bass as bass  # noqa: F401  (Bass types via bacc)
import concourse.mybir as mybir
import concourse.tile as tile
from concourse import bacc, dve_ops
from concourse.dve_spec import Spec, Src0, C0, C1, C2, C3, One, sq, _spill_c3_to_src1
from concourse.bass_utils import run_bass_kernel_spmd
from contextlib import ExitStack

f32 = mybir.dt.float32
bf16 = mybir.dt.bfloat16
AF = mybir.ActivationFunctionType
MUL = mybir.AluOpType.mult

B, S, D = 4, 2048, 1024
H = 16
DH = 64
NCORES = 8
G_HEADS = 512  # head dims per core (8 heads)

EXP_B = 1.0 / 128.0  # inner poly scale: exp(x/8) = (e^(x/128))^16
EXPP_C0 = 1.0 / (24.0 * 128.0**4)
EXPP_C1 = 1.0 / (6.0 * 128.0**3)
EXPP_C2 = 1.0 / (2.0 * 128.0**2)

# ---- custom DVE ops: two-op exp chain --------------------------------------
_t1 = Src0 * C0 + C1
_t2 = _t1 * Src0 + C2
_t3 = _t2 * Src0 + C3
_t4 = _t3 * Src0 + One


def _ref_expp(in0, in1, s0, s1, imm2):
    x = in0.astype(np.float32)
    return (((x * s0 + s1) * x + imm2) * x + in1) * x + 1.0


EXPP_ANT = dve_ops.DveOp(
    "EXPP_ANT",
    Spec(body=_spill_c3_to_src1(_t4), reference=_ref_expp),
    subdim=False,
    uops_sha={"v3": "728e43d6680666f6", "v4": "9a9d1d3477880b00"},
)


def _ref_pow16s(in0, in1, s0, s1, imm2):
    t = in0.astype(np.float32)
    t = t * t
    t = t * t
    t = t * t
    t = t * t
    return t * s0


POW16S_ANT = dve_ops.DveOp(
    "POW16S_ANT",
    Spec(body=sq(sq(sq(sq(Src0)))) * C0, reference=_ref_pow16s),
    subdim=False,
    uops_sha={"v3": "dc10736d1c0a5ecc", "v4": "4d740a20ba0e2e80"},
)

for _op in (EXPP_ANT, POW16S_ANT):
    if _op.name not in dve_ops._SUB_OPCODE_FOR_NAME:
        dve_ops.OPS.append(_op)
        dve_ops.CUSTOM_DVE_SPECS[_op.name] = _op.spec
        dve_ops._SUB_OPCODE_FOR_NAME[_op.name] = (
            max(dve_ops._SUB_OPCODE_FOR_NAME.values()) + 1
        )


def build():
    nc = bacc.Bacc(None, target_bir_lowering=False)

    # chunk-major packed inputs: row p, then [sc, d, s'] with source row
    # d*128+p, col sc*512+s' -- each [:, sc] slice is 8KB/partition contiguous
    Qp = nc.dram_tensor("Qp", [128, 4 * 8 * 512], bf16, kind="ExternalInput")
    Kp = nc.dram_tensor("Kp", [128, 4 * 8 * 512], bf16, kind="ExternalInput")
    Vp = nc.dram_tensor("Vp", [128, 4 * 8 * 512], bf16, kind="ExternalInput")
    WqT = nc.dram_tensor("WqT", [128, 8 * G_HEADS], bf16, kind="ExternalInput")
    WkT = nc.dram_tensor("WkT", [128, 8 * G_HEADS], bf16, kind="ExternalInput")
    WvT = nc.dram_tensor("WvT", [128, 8 * G_HEADS], bf16, kind="ExternalInput")
    WoT = nc.dram_tensor("WoT", [128, 4 * D], bf16, kind="ExternalInput")
    bqp = nc.dram_tensor("bqp", [128, 4], f32, kind="ExternalInput")
    bkp = nc.dram_tensor("bkp", [128, 4], f32, kind="ExternalInput")
    Y = nc.dram_tensor("Y", [S, D], bf16, kind="ExternalOutput")

    qsrc = Qp.ap().rearrange("p (sc d s) -> p sc d s", sc=4, d=8)
    ksrc = Kp.ap().rearrange("p (sc d s) -> p sc d s", sc=4, d=8)
    vsrc = Vp.ap().rearrange("p (sc d s) -> p sc d s", sc=4, d=8)

    with tile.TileContext(nc) as tc, ExitStack() as top:
        per = top.enter_context(tc.tile_pool(name="per", bufs=1))
        wq = top.enter_context(tc.tile_pool(name="wq", bufs=1))
        cst = top.enter_context(tc.tile_pool(name="cst", bufs=1))
        xqp = top.enter_context(tc.tile_pool(name="xq", bufs=2))
        ppool = top.enter_context(tc.tile_pool(name="pP", bufs=4))
        pdp = top.enter_context(tc.tile_pool(name="pd", bufs=3))
        epool = top.enter_context(tc.tile_pool(name="eE", bufs=2))
        oev = top.enter_context(tc.tile_pool(name="oev", bufs=2))
        dnp = top.enter_context(tc.tile_pool(name="dnp", bufs=2))
        rcp = top.enter_context(tc.tile_pool(name="rcp", bufs=2))
        osc = top.enter_context(tc.tile_pool(name="osc", bufs=2))
        yev = top.enter_context(tc.tile_pool(name="yev", bufs=3))
        pps = top.enter_context(tc.tile_pool(name="pps", bufs=2, space="PSUM"))
        spool = top.enter_context(tc.tile_pool(name="sS", bufs=2, space="PSUM"))
        opool = top.enter_context(tc.tile_pool(name="sO", bufs=2, space="PSUM"))

        # persistent projection results
        kT_t = [per.tile([128, S], bf16, tag=f"kT{i}", name=f"kT{i}") for i in range(4)]
        v_st = [
            per.tile([128, 8 * 65], bf16, tag=f"v{i}", name=f"v{i}") for i in range(16)
        ]
        qq_t = [
            [per.tile([128, 512], bf16, tag=f"qq{i}_{j}", name=f"qq{i}_{j}")
             for j in range(4)]
            for i in range(4)
        ]
        AT_q = [
            [per.tile([128, 512], bf16, tag=f"AT{i}_{j}", name=f"AT{i}_{j}")
             for j in range(4)]
            for i in range(4)
        ]

        bq_sb = cst.tile([128, 4], f32, tag="bq")
        bk_sb = cst.tile([128, 4], f32, tag="bk")
        cexp = cst.tile([128, 1], f32, tag="cexp")
        nc.vector.memset(cexp[:], EXP_B)
        ones_bc = cst.tile([128, 64], bf16, tag="ones_bc")
        nc.vector.memset(ones_bc[:], 1.0)
        Wk_sb = wq.tile([128, 8, G_HEADS], bf16, tag="Wk")
        Wq_sb = wq.tile([128, 8, G_HEADS], bf16, tag="Wq")
        Wv_sb = wq.tile([128, 8, G_HEADS], bf16, tag="Wv")
        Wo_sb = wq.tile([128, 4, D], bf16, tag="Wo")

        # warm the exp table set early (one-time ~2.7us load)
        warm = cst.tile([128, 8], f32, tag="warm")
        nc.vector.memset(warm[:], 0.0)
        nc.scalar.activation(warm[:], warm[:], AF.Exp)

        # ---- DMA issues -----------------------------------------------------
        # gpsimd: weights + biases (small, early)
        nc.gpsimd.dma_start(Wk_sb[:], WkT.ap().rearrange("p (d c) -> p d c", d=8))
        nc.gpsimd.dma_start(Wq_sb[:], WqT.ap().rearrange("p (d c) -> p d c", d=8))
        nc.gpsimd.dma_start(Wv_sb[:], WvT.ap().rearrange("p (d c) -> p d c", d=8))
        nc.gpsimd.dma_start(bq_sb[:], bqp[:, :])
        nc.gpsimd.dma_start(bk_sb[:], bkp[:, :])
        nc.gpsimd.dma_start(Wo_sb[:], WoT.ap().rearrange("p (g n) -> p g n", g=4))

        qx_t = {}

        def dma_qx(qp, eng):
            qx_t[qp] = xqp.tile([128, 8, 512], bf16, tag="qx", name=f"qx{qp}")
            eng.dma_start(qx_t[qp][:], qsrc[:, qp])

        # scalar queue (hwdge): first Q quarter + all four V quarters
        dma_qx(0, nc.scalar)

        with tc.tile_pool(name="kx", bufs=4) as kxp, \
             tc.tile_pool(name="vx", bufs=2) as vxp:
            kx = [kxp.tile([128, 8, 512], bf16, tag="kx", name=f"kx{sc}")
                  for sc in range(4)]
            vx = {}
            for sc in range(4):
                nc.sync.dma_start(kx[sc][:], ksrc[:, sc])
            # vx0/vx1 on the scalar hwdge queue (no waits -> no queue block);
            # vx2/vx3 reuse vx0/vx1's buffers, so their issue WAITS on the
            # free -- park them on the sync queue, which has nothing pending
            # (putting them on scalar would deadlock: exp(1) behind the
            # blocked issue, while the free transitively needs exp(1))
            for sc in range(4):
                vx[sc] = vxp.tile([128, 8, 512], bf16, tag="vx", name=f"vx{sc}")
                (nc.scalar if sc < 2 else nc.sync).dma_start(vx[sc][:], vsrc[:, sc])

            # ---- filler emitters -------------------------------------------
            def emit_kproj(sc, hp):
                ps = pps.tile([128, 512], f32, tag="ps", name=f"psk{sc}_{hp}")
                for dc in range(8):
                    nc.tensor.matmul(
                        ps[:],
                        Wk_sb[:, dc, hp * 128:(hp + 1) * 128],
                        kx[sc][:, dc, :],
                        start=(dc == 0),
                        stop=(dc == 7),
                    )
                nc.vector.tensor_scalar_add(
                    kT_t[hp][:, sc * 512:(sc + 1) * 512], ps[:], bk_sb[:, hp:hp + 1]
                )

            def emit_qproj(qp, hp):
                ps = pps.tile([128, 512], f32, tag="ps", name=f"psq{qp}_{hp}")
                for dc in range(8):
                    nc.tensor.matmul(
                        ps[:],
                        Wq_sb[:, dc, hp * 128:(hp + 1) * 128],
                        qx_t[qp][:, dc, :],
                        start=(dc == 0),
                        stop=(dc == 7),
                    )
                nc.vector.tensor_scalar_add(
                    qq_t[hp][qp][:], ps[:], bq_sb[:, hp:hp + 1]
                )

            def emit_vproj(st):
                sc, j = st // 4, st % 4
                ps = pps.tile([128, 512], f32, tag="ps", name=f"psv{st}")
                for dc in range(8):
                    nc.tensor.matmul(
                        ps[:],
                        vx[sc][:, dc, j * 128:(j + 1) * 128],
                        Wv_sb[:, dc, :],
                        start=(dc == 0),
                        stop=(dc == 7),
                    )
                vd = v_st[st][:].rearrange("p (h c) -> p h c", c=65)
                nc.vector.tensor_copy(
                    vd[:, :, 0:64], ps[:].rearrange("p (h c) -> p h c", c=64)
                )
                nc.vector.memset(vd[:, :, 64:65], 1.0)

            def emit_oproj(qp, chain):
                q4, nh = chain // 2, chain % 2
                qt_g = qp * 4 + q4
                ps = pps.tile([128, 512], f32, tag="ps", name=f"pso{qp}_{chain}")
                for hp in range(4):
                    nc.tensor.matmul(
                        ps[:],
                        AT_q[hp][qp][:, q4 * 128:(q4 + 1) * 128],
                        Wo_sb[:, hp, nh * 512:(nh + 1) * 512],
                        start=(hp == 0),
                        stop=(hp == 3),
                    )
                ye = yev.tile([128, 512], bf16, tag="ye")
                nc.vector.tensor_copy(ye[:], ps[:])
                (nc.gpsimd if chain % 2 else nc.sync).dma_start(
                    Y[qt_g * 128:(qt_g + 1) * 128, nh * 512:(nh + 1) * 512], ye[:]
                )

            # normalize pair (qp, hp): evict O, broadcast denominator via a
            # K=1 matmul, reciprocal, multiply; hloc=1 partition-shifts via DMA
            def emit_norm(qp, hp, O_t, step):
                for hloc in (0, 1) if step is None else (step,):
                    O = O_t[hloc]
                    ov = oev.tile([128, 512], f32, tag="ov")
                    nc.vector.tensor_copy(ov[0:64, :], O[0:64, :])
                    dn = dnp.tile([128, 512], bf16, tag="dn")
                    nc.vector.tensor_copy(dn[64:65, :], O[64:65, :])
                    denb = pps.tile([128, 512], f32, tag="ps", name=f"dnb{qp}{hp}{hloc}")
                    # lhsT/rhs must share base_partition -> use ones row 64
                    nc.tensor.matmul(
                        denb[0:64, :], ones_bc[64:65, :], dn[64:65, :],
                        start=True, stop=True,
                    )
                    rcb = rcp.tile([128, 512], f32, tag="rcb")
                    nc.vector.reciprocal_approx_fast(rcb[0:64, :], denb[0:64, :])
                    if hloc == 0:
                        nc.vector.tensor_tensor(
                            AT_q[hp][qp][0:64, :], ov[0:64, :], rcb[0:64, :], MUL
                        )
                    else:
                        sct = osc.tile([128, 512], bf16, tag="osc")
                        nc.vector.tensor_tensor(
                            sct[0:64, :], ov[0:64, :], rcb[0:64, :], MUL
                        )
                        nc.sync.dma_start(AT_q[hp][qp][64:128, :], sct[0:64, :])

            # ---- attention pair --------------------------------------------
            def emit_pair(i, dve_kcs, fillers, pre=None):
                qp, hp = i // 4, i % 4
                kt = kT_t[hp]
                qtile = qq_t[hp][qp]
                scalar_kcs = [k for k in range(16) if k not in dve_kcs]
                pv_seq = scalar_kcs + list(dve_kcs)
                O_t = [
                    opool.tile([128, 512], f32, tag="O", name=f"O{i}_0"),
                    opool.tile([128, 512], f32, tag="O", name=f"O{i}_1"),
                ]
                P_of = {}
                pow_pending = []  # (kc, E, Pd) to flush 2 slots later

                def emit_pv(kc):
                    Pt = P_of[kc]
                    first = kc == pv_seq[0]
                    last = kc == pv_seq[15]
                    for hloc in range(2):
                        lv = v_st[kc][:, (2 * hp + hloc) * 65:(2 * hp + hloc) * 65 + 65]
                        nc.tensor.matmul(
                            O_t[hloc][0:65, :],
                            lv,
                            Pt[:, hloc * 512:(hloc + 1) * 512],
                            start=first,
                            stop=last,
                        )

                for j in range(16):
                    # flush POW16 for the DVE tile from 2 slots ago (before
                    # this slot's pre/fillers so it leads them on the DVE queue)
                    while pow_pending and pow_pending[0][0] <= j - 2:
                        _, E, Pd = pow_pending.pop(0)
                        nc.vector._custom_dve(POW16S_ANT, out=Pd[:], in0=E[:], s0=1.0)
                    S_big = spool.tile([128, 1024], f32, tag="S", name=f"S{i}_{j}")
                    for hloc in range(2):
                        nc.tensor.matmul(
                            S_big[:, hloc * 512:(hloc + 1) * 512],
                            kt[hloc * 64:hloc * 64 + 64, j * 128:(j + 1) * 128],
                            qtile[hloc * 64:hloc * 64 + 64, :],
                            start=True,
                            stop=True,
                        )
                    if j in dve_kcs:
                        E = epool.tile([128, 1024], f32, tag="E")
                        Pd = pdp.tile([128, 1024], bf16, tag="Pd")
                        nc.vector._custom_dve(
                            EXPP_ANT, out=E[:], in0=S_big[:], in1=cexp[:],
                            s0=EXPP_C0, s1=EXPP_C1, imm2=EXPP_C2,
                        )
                        pow_pending.append((j, E, Pd))
                        P_of[j] = Pd
                    else:
                        Pt = ppool.tile([128, 1024], bf16, tag="P", name="P")
                        nc.scalar.activation(Pt[:], S_big[:], AF.Exp, scale=0.125)
                        P_of[j] = Pt
                    if pre is not None and j in pre:
                        for fn in pre[j]:
                            fn()
                    if j in fillers:
                        for fn in fillers[j]:
                            fn()
                    if j >= 3:
                        emit_pv(pv_seq[j - 3])
                while pow_pending:
                    _, E, Pd = pow_pending.pop(0)
                    nc.vector._custom_dve(POW16S_ANT, out=Pd[:], in0=E[:], s0=1.0)
                emit_pv(pv_seq[13])
                emit_pv(pv_seq[14])
                emit_pv(pv_seq[15])
                return O_t

            # ---- phase A: first K / Q projections --------------------------
            emit_kproj(0, 0)
            emit_qproj(0, 0)

            # ---- pair schedule ---------------------------------------------
            # FILL[i]: slot -> [filler closures]
            K, Q, V, Og = emit_kproj, emit_qproj, emit_vproj, emit_oproj
            norm_t = {}  # pair i -> (qp, hp, O_t) awaiting normalize

            def mknorm(i, step):
                def _n():
                    qp0, hp0, O0 = norm_t[i]
                    emit_norm(qp0, hp0, O0, step)
                return _n

            FILL = {
                0: {1: [lambda: K(1, 0)], 4: [lambda: K(2, 0)],
                    6: [lambda: K(3, 0)], 5: [lambda: K(0, 1)],
                    7: [lambda: K(1, 1)], 9: [lambda: K(2, 1)],
                    11: [lambda: K(3, 1)], 13: [lambda: Q(0, 1)]},
                1: {1: [lambda: K(0, 2)], 3: [lambda: K(1, 2)],
                    5: [lambda: K(2, 2)], 7: [lambda: K(3, 2)],
                    9: [lambda: K(0, 3)], 11: [lambda: K(1, 3)],
                    13: [lambda: K(2, 3)], 14: [lambda: K(3, 3)],
                    6: [lambda: Q(0, 2)]},
                2: {6: [lambda: Q(0, 3)]},
                3: {4: [lambda: Q(1, 0)], 10: [lambda: Q(1, 1)]},
                4: {2: [lambda: Q(1, 2)], 9: [lambda: Q(1, 3)],
                    6: [lambda: Og(0, 0)], 12: [lambda: Og(0, 1)]},
                5: {2: [lambda: Og(0, 2)], 12: [lambda: Og(0, 3)],
                    8: [lambda: Q(2, 0)]},
                6: {2: [lambda: Og(0, 4)], 10: [lambda: Og(0, 5)],
                    6: [lambda: Q(2, 1)]},
                7: {2: [lambda: Og(0, 6)], 10: [lambda: Og(0, 7)],
                    6: [lambda: Q(2, 2)]},
                8: {4: [lambda: Q(2, 3)], 6: [lambda: Og(1, 0)],
                    12: [lambda: Og(1, 1)]},
                9: {2: [lambda: Og(1, 2)], 10: [lambda: Og(1, 3)],
                    6: [lambda: Q(3, 0)]},
                10: {2: [lambda: Og(1, 4)], 10: [lambda: Og(1, 5)],
                     6: [lambda: Q(3, 1)]},
                11: {2: [lambda: Og(1, 6)], 10: [lambda: Og(1, 7)],
                     6: [lambda: Q(3, 2)]},
                12: {4: [lambda: Q(3, 3)], 6: [lambda: Og(2, 0)],
                     12: [lambda: Og(2, 1)]},
                13: {4: [lambda: Og(2, 2)], 10: [lambda: Og(2, 3)]},
                14: {4: [lambda: Og(2, 4)], 10: [lambda: Og(2, 5)]},
                15: {4: [lambda: Og(2, 6)], 10: [lambda: Og(2, 7)]},
            }

            for i in range(16):
                qp, hp = i // 4, i % 4
                dve_kcs = () if i < 2 else (0, 1)
                fillers = dict(FILL[i])
                if i == 0:
                    # V projections ride every slot of pair 0, just after
                    # that slot's scores (before its PV consumes v_st)
                    for j in range(16):
                        fillers.setdefault(j, [])
                        fillers[j] = [lambda st=j: V(st)] + fillers[j]
                # prev pair's normalize lands in slots 2-3 of this pair --
                # AFTER this pair's EXPP(0)/EXPP(1) on the DVE queue, else
                # the norm chain delays EXPP(1) past scores(3)'s S-slot reuse
                pre = None
                if i > 0:
                    pre = {2: [mknorm(i - 1, 0)], 3: [mknorm(i - 1, 1)]}
                # qx prefetches on the gpsimd queue
                if i == 1:
                    dma_qx(1, nc.gpsimd)
                elif i == 5:
                    dma_qx(2, nc.gpsimd)
                elif i == 7:
                    dma_qx(3, nc.gpsimd)
                O_t = emit_pair(i, dve_kcs, fillers, pre=pre)
                norm_t[i] = (qp, hp, O_t)

        # ---- tail: last pair's normalize + final out-proj chains -----------
        emit_norm(3, 3, norm_t[15][2], None)
        for chain in range(8):
            emit_oproj(3, chain)

    nc.compile()
    return nc


_NC = None


def _get_nc():
    global _NC
    if _NC is None:
        _NC = build()
    return _NC


def _wpack(WT):
    # [D, C] -> [128, 8*C] with row p holding [WT[dc*128+p, :] for dc in 0..8]
    D_, C = WT.shape
    d = D_ // 128
    return WT.reshape(d, 128, C).transpose(1, 0, 2).reshape(128, d * C)


def _xpack(Xb, bf):
    # [S, D] -> [128, 4*8*512]: row p holds [X.T[d*128+p, sc*512:(sc+1)*512]
    # for sc in 0..4 for d in 0..8] (chunk-major, contiguous 8KB DMA slices)
    XT = Xb.T  # [D, S]
    Hm = XT.reshape(8, 128, 4, 512)  # [d, p, sc, s']
    return np.ascontiguousarray(
        Hm.transpose(1, 2, 0, 3).astype(bf).reshape(128, 4 * 8 * 512)
    )


def _prep_core(Q, K, V, Wq, bq, Wk, bk, Wv, Wo, b, g):
    c = np.ascontiguousarray
    bf = ml_dtypes.bfloat16
    hs = slice(g * G_HEADS, (g + 1) * G_HEADS)
    return {
        "Qp": _xpack(Q[b], bf),
        "Kp": _xpack(K[b], bf),
        "Vp": _xpack(V[b], bf),
        "WqT": c(_wpack(Wq[hs, :].T.astype(bf))),
        "WkT": c(_wpack(Wk[hs, :].T.astype(bf))),
        "WvT": c(_wpack(Wv[hs, :].T.astype(bf))),
        "WoT": c(_wpack(Wo[:, hs].T.astype(bf))),
        "bqp": c(bq[hs].reshape(4, 128).T),
        "bkp": c(bk[hs].reshape(4, 128).T),
    }


def kernel(Q, K, V, Wq, bq, Wk, bk, Wv, bv, Wo, bo, _want_trace=False):
    Q, K, V = (np.asarray(x, np.float32) for x in (Q, K, V))
    Wq, bq, Wk, bk, Wv, bv, Wo, bo = (
        np.asarray(x, np.float32) for x in (Wq, bq, Wk, bk, Wv, bv, Wo, bo)
    )
    nc = _get_nc()
    in_maps = [
        _prep_core(Q, K, V, Wq, bq, Wk, bk, Wv, Wo, b=c % 4, g=c // 4)
        for c in range(NCORES)
    ]
    res = run_bass_kernel_spmd(
        nc, in_maps, core_ids=list(range(NCORES)), trace=_want_trace
    )
    out = np.zeros((B, S, D), np.float32)
    for c in range(NCORES):
        out[c % 4] += res.results[c]["Y"].astype(np.float32)
    out += (bo + Wo.astype(np.float64) @ bv.astype(np.float64)).astype(np.float32)[
        None, None, :
    ]
    if _want_trace:
        kernel.last_exec_time_ns = res.exec_time_ns
        kernel.last_trace = res.instructions_and_trace
    return out


# revision 14
# speedup vs baseline: 1.1602x; 1.1602x over previous
"""Multi-head attention (B=4, S=2048, D=1024, H=16) on 8 Trainium2 cores.

Sharding: DP=4 over batch x TP=2 over heads (8 heads/core).

Single fused pipeline per core, built so the tensor engine never idles:
  - inputs are HOST-PACKED chunk-major ([128, sc, d, s'] with row index
    d*128+p) so every input DMA is 128 x 8KB contiguous descriptors
  - attention runs as 16 pairs (qp quarter x hp head-pair); ALL projection
    work (K/V/Q/O) is emitted as "filler" matmul chains inside the pairs'
    kc slots, sized so per-slot tensor time (~1.1-1.3us) stays above the
    scalar engine's exp time (~1.1us) -- the scalar engine never gates.
  - scores S^T = kT.T @ qq per 128-key chunk; P = exp(S/8) mostly on the
    scalar engine's exp LUT; per pair, kc=0,1 run on the vector engine via
    a custom 2-op chain (deg-4 Taylor poly of e^(x/128), then ^16) and
    their PV accumulation is rotated to the END of the pair so the DVE
    FIFO latency is hidden (PV order = [2..15, 0, 1]).
  - O^T = [v | 1].T @ P (ones column = softmax denominator in row 64).
  - normalize WITHOUT a DRAM round-trip: denominator row is evicted to
    bf16, broadcast to 64 partitions by a K=1 matmul (ones lhsT), then
    reciprocal_approx_fast + multiply on the vector engine. The hloc=1
    half is partition-shifted into AT[64:128] by an SBUF->SBUF DMA.
  - output projection Y_g = A_g @ Wo_g.T chains are fillers too; Y bf16.
Host sums the two TP partials per batch and adds bo + Wo @ bv (the v-bias
commutes through the normalized softmax).
"""

import os
import sys

sys.path.insert(0, "/opt/trn_rl_repo")
os.environ.setdefault("MYCRO_LOCAL_CACHE", "1")

import numpy as np
import ml_dtypes
import concourse.bass as bass  # noqa: F401  (Bass types via bacc)
import concourse.mybir as mybir
import concourse.tile as tile
from concourse import bacc, dve_ops
from concourse.dve_spec import Spec, Src0, C0, C1, C2, C3, One, sq, _spill_c3_to_src1
from concourse.bass_utils import run_bass_kernel_spmd
from contextlib import ExitStack

f32 = mybir.dt.float32
bf16 = mybir.dt.bfloat16
AF = mybir.ActivationFunctionType
MUL = mybir.AluOpType.mult

B, S, D = 4, 2048, 1024
H = 16
DH = 64
NCORES = 8
G_HEADS = 512  # head dims per core (8 heads)

EXP_B = 1.0 / 128.0  # inner poly scale: exp(x/8) = (e^(x/128))^16
EXPP_C0 = 1.0 / (24.0 * 128.0**4)
EXPP_C1 = 1.0 / (6.0 * 128.0**3)
EXPP_C2 = 1.0 / (2.0 * 128.0**2)

# ---- custom DVE ops: two-op exp chain --------------------------------------
_t1 = Src0 * C0 + C1
_t2 = _t1 * Src0 + C2
_t3 = _t2 * Src0 + C3
_t4 = _t3 * Src0 + One


def _ref_expp(in0, in1, s0, s1, imm2):
    x = in0.astype(np.float32)
    return (((x * s0 + s1) * x + imm2) * x + in1) * x + 1.0


EXPP_ANT = dve_ops.DveOp(
    "EXPP_ANT",
    Spec(body=_spill_c3_to_src1(_t4), reference=_ref_expp),
    subdim=False,
    uops_sha={"v3": "728e43d6680666f6", "v4": "9a9d1d3477880b00"},
)


def _ref_pow16s(in0, in1, s0, s1, imm2):
    t = in0.astype(np.float32)
    t = t * t
    t = t * t
    t = t * t
    t = t * t
    return t * s0


POW16S_ANT = dve_ops.DveOp(
    "POW16S_ANT",
    Spec(body=sq(sq(sq(sq(Src0)))) * C0, reference=_ref_pow16s),
    subdim=False,
    uops_sha={"v3": "dc10736d1c0a5ecc", "v4": "4d740a20ba0e2e80"},
)

for _op in (EXPP_ANT, POW16S_ANT):
    if _op.name not in dve_ops._SUB_OPCODE_FOR_NAME:
        dve_ops.OPS.append(_op)
        dve_ops.CUSTOM_DVE_SPECS[_op.name] = _op.spec
        dve_ops._SUB_OPCODE_FOR_NAME[_op.name] = (
            max(dve_ops._SUB_OPCODE_FOR_NAME.values()) + 1
        )


def build():
    nc = bacc.Bacc(None, target_bir_lowering=False)

    # chunk-major packed inputs: row p, then [sc, d, s'] with source row
    # d*128+p, col sc*512+s' -- each [:, sc] slice is 8KB/partition contiguous
    Qp = nc.dram_tensor("Qp", [128, 4 * 8 * 512], bf16, kind="ExternalInput")
    Kp = nc.dram_tensor("Kp", [128, 4 * 8 * 512], bf16, kind="ExternalInput")
    Vp = nc.dram_tensor("Vp", [128, 4 * 8 * 512], bf16, kind="ExternalInput")
    WqT = nc.dram_tensor("WqT", [128, 8 * G_HEADS], bf16, kind="ExternalInput")
    WkT = nc.dram_tensor("WkT", [128, 8 * G_HEADS], bf16, kind="ExternalInput")
    WvT = nc.dram_tensor("WvT", [128, 8 * G_HEADS], bf16, kind="ExternalInput")
    WoT = nc.dram_tensor("WoT", [128, 4 * D], bf16, kind="ExternalInput")
    bqp = nc.dram_tensor("bqp", [128, 4], f32, kind="ExternalInput")
    bkp = nc.dram_tensor("bkp", [128, 4], f32, kind="ExternalInput")
    Y = nc.dram_tensor("Y", [S, D], bf16, kind="ExternalOutput")

    qsrc = Qp.ap().rearrange("p (sc d s) -> p sc d s", sc=4, d=8)
    ksrc = Kp.ap().rearrange("p (sc d s) -> p sc d s", sc=4, d=8)
    vsrc = Vp.ap().rearrange("p (sc d s) -> p sc d s", sc=4, d=8)

    with tile.TileContext(nc) as tc, ExitStack() as top:
        per = top.enter_context(tc.tile_pool(name="per", bufs=1))
        wq = top.enter_context(tc.tile_pool(name="wq", bufs=1))
        cst = top.enter_context(tc.tile_pool(name="cst", bufs=1))
        xqp = top.enter_context(tc.tile_pool(name="xq", bufs=2))
        ppool = top.enter_context(tc.tile_pool(name="pP", bufs=4))
        oev = top.enter_context(tc.tile_pool(name="oev", bufs=2))
        dnp = top.enter_context(tc.tile_pool(name="dnp", bufs=2))
        rcp = top.enter_context(tc.tile_pool(name="rcp", bufs=2))
        osc = top.enter_context(tc.tile_pool(name="osc", bufs=2))
        yev = top.enter_context(tc.tile_pool(name="yev", bufs=3))
        pps = top.enter_context(tc.tile_pool(name="pps", bufs=2, space="PSUM"))
        spool = top.enter_context(tc.tile_pool(name="sS", bufs=2, space="PSUM"))
        opool = top.enter_context(tc.tile_pool(name="sO", bufs=2, space="PSUM"))

        # persistent projection results
        kT_t = [per.tile([128, S], bf16, tag=f"kT{i}", name=f"kT{i}") for i in range(4)]
        v_st = [
            per.tile([128, 8 * 65], bf16, tag=f"v{i}", name=f"v{i}") for i in range(16)
        ]
        qq_t = [
            [per.tile([128, 512], bf16, tag=f"qq{i}_{j}", name=f"qq{i}_{j}")
             for j in range(4)]
            for i in range(4)
        ]
        AT_q = [
            [per.tile([128, 512], bf16, tag=f"AT{i}_{j}", name=f"AT{i}_{j}")
             for j in range(4)]
            for i in range(4)
        ]

        bq_sb = cst.tile([128, 4], f32, tag="bq")
        bk_sb = cst.tile([128, 4], f32, tag="bk")
        cexp = cst.tile([128, 1], f32, tag="cexp")
        nc.vector.memset(cexp[:], EXP_B)
        ones_bc = cst.tile([128, 64], bf16, tag="ones_bc")
        nc.vector.memset(ones_bc[:], 1.0)
        Wk_sb = wq.tile([128, 8, G_HEADS], bf16, tag="Wk")
        Wq_sb = wq.tile([128, 8, G_HEADS], bf16, tag="Wq")
        Wv_sb = wq.tile([128, 8, G_HEADS], bf16, tag="Wv")
        Wo_sb = wq.tile([128, 4, D], bf16, tag="Wo")

        # warm the exp table set early (one-time ~2.7us load)
        warm = cst.tile([128, 8], f32, tag="warm")
        nc.vector.memset(warm[:], 0.0)
        nc.scalar.activation(warm[:], warm[:], AF.Exp)

        # ---- DMA issues -----------------------------------------------------
        # per-ring DMA bandwidth is ~110GB/s, so spread the ~11MB of input
        # across all three rings (sync, scalar-hwdge, gpsimd-swdge) in
        # consumption order. The scalar queue gets ONLY first-use buffers:
        # a buffer-reusing issue would park a WAR wait in front of every
        # queued exp. vx2/vx3 (which reuse vx0/vx1) go on sync, whose later
        # work (AT shifts) is far away.
        qx_t = {}

        def dma_qx(qp, eng):
            qx_t[qp] = xqp.tile([128, 8, 512], bf16, tag="qx", name=f"qx{qp}")
            eng.dma_start(qx_t[qp][:], qsrc[:, qp])

        nc.gpsimd.dma_start(bq_sb[:], bqp[:, :])
        nc.gpsimd.dma_start(bk_sb[:], bkp[:, :])
        nc.gpsimd.dma_start(Wk_sb[:], WkT.ap().rearrange("p (d c) -> p d c", d=8))
        nc.scalar.dma_start(Wq_sb[:], WqT.ap().rearrange("p (d c) -> p d c", d=8))
        nc.gpsimd.dma_start(Wv_sb[:], WvT.ap().rearrange("p (d c) -> p d c", d=8))
        dma_qx(0, nc.scalar)

        with tc.tile_pool(name="kx", bufs=4) as kxp, \
             tc.tile_pool(name="vx", bufs=2) as vxp:
            kx = [kxp.tile([128, 8, 512], bf16, tag="kx", name=f"kx{sc}")
                  for sc in range(4)]
            vx = {}
            for sc in range(4):
                vx[sc] = vxp.tile([128, 8, 512], bf16, tag="vx", name=f"vx{sc}")
            nc.sync.dma_start(kx[0][:], ksrc[:, 0])
            nc.gpsimd.dma_start(vx[0][:], vsrc[:, 0])
            nc.scalar.dma_start(kx[1][:], ksrc[:, 1])
            nc.sync.dma_start(vx[1][:], vsrc[:, 1])
            nc.gpsimd.dma_start(
                Wo_sb[:], WoT.ap().rearrange("p (g n) -> p g n", g=4)
            )
            nc.sync.dma_start(kx[2][:], ksrc[:, 2])
            nc.gpsimd.dma_start(kx[3][:], ksrc[:, 3])
            nc.sync.dma_start(vx[2][:], vsrc[:, 2])
            nc.sync.dma_start(vx[3][:], vsrc[:, 3])

            # ---- filler emitters -------------------------------------------
            def emit_kproj(sc, hp):
                ps = pps.tile([128, 512], f32, tag="ps", name=f"psk{sc}_{hp}")
                for dc in range(8):
                    nc.tensor.matmul(
                        ps[:],
                        Wk_sb[:, dc, hp * 128:(hp + 1) * 128],
                        kx[sc][:, dc, :],
                        start=(dc == 0),
                        stop=(dc == 7),
                    )
                nc.vector.tensor_scalar_add(
                    kT_t[hp][:, sc * 512:(sc + 1) * 512], ps[:], bk_sb[:, hp:hp + 1]
                )

            def emit_qproj(qp, hp):
                ps = pps.tile([128, 512], f32, tag="ps", name=f"psq{qp}_{hp}")
                for dc in range(8):
                    nc.tensor.matmul(
                        ps[:],
                        Wq_sb[:, dc, hp * 128:(hp + 1) * 128],
                        qx_t[qp][:, dc, :],
                        start=(dc == 0),
                        stop=(dc == 7),
                    )
                nc.vector.tensor_scalar_add(
                    qq_t[hp][qp][:], ps[:], bq_sb[:, hp:hp + 1]
                )

            def emit_vproj(st):
                sc, j = st // 4, st % 4
                ps = pps.tile([128, 512], f32, tag="ps", name=f"psv{st}")
                for dc in range(8):
                    nc.tensor.matmul(
                        ps[:],
                        vx[sc][:, dc, j * 128:(j + 1) * 128],
                        Wv_sb[:, dc, :],
                        start=(dc == 0),
                        stop=(dc == 7),
                    )
                vd = v_st[st][:].rearrange("p (h c) -> p h c", c=65)
                nc.vector.tensor_copy(
                    vd[:, :, 0:64], ps[:].rearrange("p (h c) -> p h c", c=64)
                )
                nc.vector.memset(vd[:, :, 64:65], 1.0)

            def emit_oproj(qp, chain):
                q4, nh = chain // 2, chain % 2
                qt_g = qp * 4 + q4
                ps = pps.tile([128, 512], f32, tag="ps", name=f"pso{qp}_{chain}")
                for hp in range(4):
                    nc.tensor.matmul(
                        ps[:],
                        AT_q[hp][qp][:, q4 * 128:(q4 + 1) * 128],
                        Wo_sb[:, hp, nh * 512:(nh + 1) * 512],
                        start=(hp == 0),
                        stop=(hp == 3),
                    )
                ye = yev.tile([128, 512], bf16, tag="ye")
                nc.vector.tensor_copy(ye[:], ps[:])
                (nc.gpsimd if chain % 2 else nc.sync).dma_start(
                    Y[qt_g * 128:(qt_g + 1) * 128, nh * 512:(nh + 1) * 512], ye[:]
                )

            # normalize pair (qp, hp): evict O, broadcast denominator via a
            # K=1 matmul, reciprocal, multiply; hloc=1 partition-shifts via DMA
            def emit_norm(qp, hp, O_t, step):
                for hloc in (0, 1) if step is None else (step,):
                    O = O_t[hloc]
                    ov = oev.tile([128, 512], f32, tag="ov")
                    nc.vector.tensor_copy(ov[0:64, :], O[0:64, :])
                    dn = dnp.tile([128, 512], bf16, tag="dn")
                    nc.vector.tensor_copy(dn[64:65, :], O[64:65, :])
                    denb = pps.tile([128, 512], f32, tag="ps", name=f"dnb{qp}{hp}{hloc}")
                    # lhsT/rhs must share base_partition -> use ones row 64
                    nc.tensor.matmul(
                        denb[0:64, :], ones_bc[64:65, :], dn[64:65, :],
                        start=True, stop=True,
                    )
                    rcb = rcp.tile([128, 512], f32, tag="rcb")
                    nc.vector.reciprocal_approx_fast(rcb[0:64, :], denb[0:64, :])
                    if hloc == 0:
                        nc.vector.tensor_tensor(
                            AT_q[hp][qp][0:64, :], ov[0:64, :], rcb[0:64, :], MUL
                        )
                    else:
                        sct = osc.tile([128, 512], bf16, tag="osc")
                        nc.vector.tensor_tensor(
                            sct[0:64, :], ov[0:64, :], rcb[0:64, :], MUL
                        )
                        nc.sync.dma_start(AT_q[hp][qp][64:128, :], sct[0:64, :])

            # ---- attention pair --------------------------------------------
            # Boundary discipline: the PREVIOUS pair's last 3 PVs and one
            # filler chain are emitted BETWEEN this pair's first scores, so
            # the tensor queue has work while exp(0)/exp(1) free the 2-deep
            # S-PSUM rotation (without it, every pair start stalls ~2us).
            def emit_pair(i, fillers, boundary, prev_tail, pre, post=()):
                qp, hp = i // 4, i % 4
                kt = kT_t[hp]
                qtile = qq_t[hp][qp]
                O_t = [
                    opool.tile([128, 512], f32, tag="O", name=f"O{i}_0"),
                    opool.tile([128, 512], f32, tag="O", name=f"O{i}_1"),
                ]
                P_of = {}

                def emit_pv(kc):
                    Pt = P_of[kc]
                    for hloc in range(2):
                        lv = v_st[kc][:, (2 * hp + hloc) * 65:(2 * hp + hloc) * 65 + 65]
                        nc.tensor.matmul(
                            O_t[hloc][0:65, :],
                            lv,
                            Pt[:, hloc * 512:(hloc + 1) * 512],
                            start=(kc == 0),
                            stop=(kc == 15),
                        )

                for j in range(16):
                    S_big = spool.tile([128, 1024], f32, tag="S", name=f"S{i}_{j}")
                    for hloc in range(2):
                        nc.tensor.matmul(
                            S_big[:, hloc * 512:(hloc + 1) * 512],
                            kt[hloc * 64:hloc * 64 + 64, j * 128:(j + 1) * 128],
                            qtile[hloc * 64:hloc * 64 + 64, :],
                            start=True,
                            stop=True,
                        )
                    Pt = ppool.tile([128, 1024], bf16, tag="P", name="P")
                    nc.scalar.activation(Pt[:], S_big[:], AF.Exp, scale=0.125)
                    P_of[j] = Pt
                    if j == 0 and prev_tail:
                        prev_tail[0]()
                    elif j == 1:
                        if prev_tail:
                            prev_tail[1]()
                            prev_tail[2]()
                        for fn in boundary:
                            fn()
                    if pre is not None and j in pre:
                        for fn in pre[j]:
                            fn()
                    if j in fillers:
                        for fn in fillers[j]:
                            fn()
                    if j >= 3:
                        emit_pv(j - 3)
                for fn in post:
                    fn()
                return O_t, [lambda: emit_pv(13), lambda: emit_pv(14),
                             lambda: emit_pv(15)]

            # ---- phase A: first K / Q projections --------------------------
            emit_kproj(0, 0)
            emit_qproj(0, 0)

            # ---- pair schedule ---------------------------------------------
            # FILL[i]: slot -> [filler closures]; BND[i]: boundary fillers
            K, Q, V, Og = emit_kproj, emit_qproj, emit_vproj, emit_oproj
            norm_t = {}  # pair i -> (qp, hp, O_t) awaiting normalize

            def mknorm(i, step):
                def _n():
                    qp0, hp0, O0 = norm_t[i]
                    emit_norm(qp0, hp0, O0, step)
                return _n

            FILL = {
                0: {3: [lambda: K(1, 0)], 5: [lambda: K(2, 0)],
                    7: [lambda: K(3, 0)], 9: [lambda: K(0, 1)],
                    10: [lambda: K(1, 1)], 11: [lambda: K(2, 1)],
                    12: [lambda: K(3, 1)], 13: [lambda: Q(0, 1)]},
                1: {1: [lambda: K(1, 2)], 3: [lambda: K(2, 2)],
                    5: [lambda: K(3, 2)], 7: [lambda: K(0, 3)],
                    9: [lambda: K(1, 3)], 11: [lambda: K(2, 3)],
                    13: [lambda: K(3, 3)], 6: [lambda: Q(0, 2)]},
                2: {},
                3: {8: [lambda: Q(1, 1)]},
                4: {6: [lambda: Og(0, 0)], 10: [lambda: Og(0, 1)],
                    13: [lambda: Q(1, 3)]},
                5: {6: [lambda: Q(2, 0)], 10: [lambda: Og(0, 3)]},
                6: {6: [lambda: Q(2, 1)], 10: [lambda: Og(0, 5)]},
                7: {6: [lambda: Q(2, 2)], 10: [lambda: Og(0, 7)]},
                8: {6: [lambda: Og(1, 0)], 10: [lambda: Og(1, 1)]},
                9: {6: [lambda: Q(3, 0)], 10: [lambda: Og(1, 3)]},
                10: {6: [lambda: Q(3, 1)], 10: [lambda: Og(1, 5)]},
                11: {6: [lambda: Q(3, 2)], 10: [lambda: Og(1, 7)]},
                12: {6: [lambda: Og(2, 0)], 10: [lambda: Og(2, 1)]},
                13: {8: [lambda: Og(2, 3)]},
                14: {8: [lambda: Og(2, 5)]},
                15: {8: [lambda: Og(2, 7)]},
            }
            BND = {
                1: [lambda: K(0, 2)],
                2: [lambda: Q(0, 3)],
                3: [lambda: Q(1, 0)],
                4: [lambda: Q(1, 2)],
                5: [lambda: Og(0, 2)],
                6: [lambda: Og(0, 4)],
                7: [lambda: Og(0, 6)],
                8: [lambda: Q(2, 3)],
                9: [lambda: Og(1, 2)],
                10: [lambda: Og(1, 4)],
                11: [lambda: Og(1, 6)],
                12: [lambda: Q(3, 3)],
                13: [lambda: Og(2, 2)],
                14: [lambda: Og(2, 4)],
                15: [lambda: Og(2, 6)],
            }

            prev_tail = None
            for i in range(16):
                qp, hp = i // 4, i % 4
                fillers = dict(FILL[i])
                post = ()
                if i == 0:
                    # V projections ride pair-0 slots (st at slot st+2, one
                    # slot ahead of PV(st) at slot st+3); V(14)/V(15) land
                    # right after slot 15, before the tail PVs fire at the
                    # pair-1 boundary
                    for st in range(14):
                        fillers.setdefault(st + 2, [])
                        fillers[st + 2] = [lambda s=st: V(s)] + fillers[st + 2]
                    post = (lambda: V(14), lambda: V(15))
                # prev pair's normalize lands in slots 2-3 of this pair
                pre = None
                if i > 0:
                    pre = {2: [mknorm(i - 1, 0)], 3: [mknorm(i - 1, 1)]}
                # qx prefetches on the gpsimd queue
                if i == 1:
                    dma_qx(1, nc.gpsimd)
                elif i == 5:
                    dma_qx(2, nc.gpsimd)
                elif i == 7:
                    dma_qx(3, nc.gpsimd)
                O_t, tail = emit_pair(
                    i, fillers, BND.get(i, []), prev_tail, pre, post=post
                )
                norm_t[i] = (qp, hp, O_t)
                prev_tail = tail

        # ---- tail: last pair's PVs + normalize + final out-proj chains -----
        for fn in prev_tail:
            fn()
        emit_norm(3, 3, norm_t[15][2], None)
        for chain in range(8):
            emit_oproj(3, chain)

    nc.compile()
    return nc


_NC = None


def _get_nc():
    global _NC
    if _NC is None:
        _NC = build()
    return _NC


def _wpack(WT):
    # [D, C] -> [128, 8*C] with row p holding [WT[dc*128+p, :] for dc in 0..8]
    D_, C = WT.shape
    d = D_ // 128
    return WT.reshape(d, 128, C).transpose(1, 0, 2).reshape(128, d * C)


def _xpack(Xb, bf):
    # [S, D] -> [128, 4*8*512]: row p holds [X.T[d*128+p, sc*512:(sc+1)*512]
    # for sc in 0..4 for d in 0..8] (chunk-major, contiguous 8KB DMA slices)
    XT = Xb.T  # [D, S]
    Hm = XT.reshape(8, 128, 4, 512)  # [d, p, sc, s']
    return np.ascontiguousarray(
        Hm.transpose(1, 2, 0, 3).astype(bf).reshape(128, 4 * 8 * 512)
    )


def _prep_core(Q, K, V, Wq, bq, Wk, bk, Wv, Wo, b, g):
    c = np.ascontiguousarray
    bf = ml_dtypes.bfloat16
    hs = slice(g * G_HEADS, (g + 1) * G_HEADS)
    return {
        "Qp": _xpack(Q[b], bf),
        "Kp": _xpack(K[b], bf),
        "Vp": _xpack(V[b], bf),
        "WqT": c(_wpack(Wq[hs, :].T.astype(bf))),
        "WkT": c(_wpack(Wk[hs, :].T.astype(bf))),
        "WvT": c(_wpack(Wv[hs, :].T.astype(bf))),
        "WoT": c(_wpack(Wo[:, hs].T.astype(bf))),
        "bqp": c(bq[hs].reshape(4, 128).T),
        "bkp": c(bk[hs].reshape(4, 128).T),
    }


def kernel(Q, K, V, Wq, bq, Wk, bk, Wv, bv, Wo, bo, _want_trace=False):
    Q, K, V = (np.asarray(x, np.float32) for x in (Q, K, V))
    Wq, bq, Wk, bk, Wv, bv, Wo, bo = (
        np.asarray(x, np.float32) for x in (Wq, bq, Wk, bk, Wv, bv, Wo, bo)
    )
    nc = _get_nc()
    in_maps = [
        _prep_core(Q, K, V, Wq, bq, Wk, bk, Wv, Wo, b=c % 4, g=c // 4)
        for c in range(NCORES)
    ]
    res = run_bass_kernel_spmd(
        nc, in_maps, core_ids=list(range(NCORES)), trace=_want_trace
    )
    out = np.zeros((B, S, D), np.float32)
    for c in range(NCORES):
        out[c % 4] += res.results[c]["Y"].astype(np.float32)
    out += (bo + Wo.astype(np.float64) @ bv.astype(np.float64)).astype(np.float32)[
        None, None, :
    ]
    if _want_trace:
        kernel.last_exec_time_ns = res.exec_time_ns
        kernel.last_trace = res.instructions_and_trace
    return out


# revision 21
# speedup vs baseline: 1.1837x; 1.0203x over previous
"""Multi-head attention (B=4, S=2048, D=1024, H=16) on 8 Trainium2 cores.

Sharding: DP=4 over batch x TP=2 over heads (8 heads/core).

Single fused pipeline per core, built so the tensor engine never idles:
  - inputs are HOST-PACKED chunk-major ([128, sc, d, s'] with row index
    d*128+p) so every input DMA is 128 x 8KB contiguous descriptors
  - attention runs as 16 pairs (qp quarter x hp head-pair); ALL projection
    work (K/V/Q/O) is emitted as "filler" matmul chains inside the pairs'
    kc slots, sized so per-slot tensor time (~1.1-1.3us) stays above the
    scalar engine's exp time (~1.1us) -- the scalar engine never gates.
  - scores S^T = kT.T @ qq per 128-key chunk; P = exp(S/8) mostly on the
    scalar engine's exp LUT; per pair, kc=0,1 run on the vector engine via
    a custom 2-op chain (deg-4 Taylor poly of e^(x/128), then ^16) and
    their PV accumulation is rotated to the END of the pair so the DVE
    FIFO latency is hidden (PV order = [2..15, 0, 1]).
  - O^T = [v | 1].T @ P (ones column = softmax denominator in row 64).
  - normalize WITHOUT a DRAM round-trip: denominator row is evicted to
    bf16, broadcast to 64 partitions by a K=1 matmul (ones lhsT), then
    reciprocal_approx_fast + multiply on the vector engine. The hloc=1
    half is partition-shifted into AT[64:128] by an SBUF->SBUF DMA.
  - output projection Y_g = A_g @ Wo_g.T chains are fillers too; Y bf16.
Host sums the two TP partials per batch and adds bo + Wo @ bv (the v-bias
commutes through the normalized softmax).
"""

import os
import sys

sys.path.insert(0, "/opt/trn_rl_repo")
os.environ.setdefault("MYCRO_LOCAL_CACHE", "1")

import numpy as np
import ml_dtypes
import concourse.bass as bass  # noqa: F401  (Bass types via bacc)
import concourse.mybir as mybir
import concourse.tile as tile
from concourse import bacc, dve_ops
from concourse.dve_spec import Spec, Src0, C0, C1, C2, C3, One, sq, _spill_c3_to_src1
from concourse.bass_utils import run_bass_kernel_spmd
from contextlib import ExitStack

f32 = mybir.dt.float32
bf16 = mybir.dt.bfloat16
AF = mybir.ActivationFunctionType
MUL = mybir.AluOpType.mult

B, S, D = 4, 2048, 1024
H = 16
DH = 64
NCORES = 8
G_HEADS = 512  # head dims per core (8 heads)

EXP_B = 1.0 / 128.0  # inner poly scale: exp(x/8) = (e^(x/128))^16
EXPP_C0 = 1.0 / (24.0 * 128.0**4)
EXPP_C1 = 1.0 / (6.0 * 128.0**3)
EXPP_C2 = 1.0 / (2.0 * 128.0**2)

# ---- custom DVE ops: two-op exp chain --------------------------------------
_t1 = Src0 * C0 + C1
_t2 = _t1 * Src0 + C2
_t3 = _t2 * Src0 + C3
_t4 = _t3 * Src0 + One


def _ref_expp(in0, in1, s0, s1, imm2):
    x = in0.astype(np.float32)
    return (((x * s0 + s1) * x + imm2) * x + in1) * x + 1.0


EXPP_ANT = dve_ops.DveOp(
    "EXPP_ANT",
    Spec(body=_spill_c3_to_src1(_t4), reference=_ref_expp),
    subdim=False,
    uops_sha={"v3": "728e43d6680666f6", "v4": "9a9d1d3477880b00"},
)


def _ref_pow16s(in0, in1, s0, s1, imm2):
    t = in0.astype(np.float32)
    t = t * t
    t = t * t
    t = t * t
    t = t * t
    return t * s0


POW16S_ANT = dve_ops.DveOp(
    "POW16S_ANT",
    Spec(body=sq(sq(sq(sq(Src0)))) * C0, reference=_ref_pow16s),
    subdim=False,
    uops_sha={"v3": "dc10736d1c0a5ecc", "v4": "4d740a20ba0e2e80"},
)

for _op in (EXPP_ANT, POW16S_ANT):
    if _op.name not in dve_ops._SUB_OPCODE_FOR_NAME:
        dve_ops.OPS.append(_op)
        dve_ops.CUSTOM_DVE_SPECS[_op.name] = _op.spec
        dve_ops._SUB_OPCODE_FOR_NAME[_op.name] = (
            max(dve_ops._SUB_OPCODE_FOR_NAME.values()) + 1
        )


def build():
    nc = bacc.Bacc(None, target_bir_lowering=False)

    # chunk-major packed inputs: row p, then [sc, d, s'] with source row
    # d*128+p, col sc*512+s' -- each [:, sc] slice is 8KB/partition contiguous
    Qp = nc.dram_tensor("Qp", [128, 4 * 8 * 512], bf16, kind="ExternalInput")
    Kp = nc.dram_tensor("Kp", [128, 4 * 8 * 512], bf16, kind="ExternalInput")
    Vp = nc.dram_tensor("Vp", [128, 4 * 8 * 512], bf16, kind="ExternalInput")
    WqT = nc.dram_tensor("WqT", [128, 8 * G_HEADS], bf16, kind="ExternalInput")
    WkT = nc.dram_tensor("WkT", [128, 8 * G_HEADS], bf16, kind="ExternalInput")
    WvT = nc.dram_tensor("WvT", [128, 8 * G_HEADS], bf16, kind="ExternalInput")
    WoT = nc.dram_tensor("WoT", [128, 4 * D], bf16, kind="ExternalInput")
    bqp = nc.dram_tensor("bqp", [128, 4], f32, kind="ExternalInput")
    bkp = nc.dram_tensor("bkp", [128, 4], f32, kind="ExternalInput")
    Y = nc.dram_tensor("Y", [S, D], bf16, kind="ExternalOutput")

    qsrc = Qp.ap().rearrange("p (sc d s) -> p sc d s", sc=4, d=8)
    ksrc = Kp.ap().rearrange("p (sc d s) -> p sc d s", sc=4, d=8)
    vsrc = Vp.ap().rearrange("p (sc d s) -> p sc d s", sc=4, d=8)

    with tile.TileContext(nc) as tc, ExitStack() as top:
        per = top.enter_context(tc.tile_pool(name="per", bufs=1))
        wq = top.enter_context(tc.tile_pool(name="wq", bufs=1))
        cst = top.enter_context(tc.tile_pool(name="cst", bufs=1))
        xqp = top.enter_context(tc.tile_pool(name="xq", bufs=2))
        ppool = top.enter_context(tc.tile_pool(name="pP", bufs=4))
        oev = top.enter_context(tc.tile_pool(name="oev", bufs=2))
        dnp = top.enter_context(tc.tile_pool(name="dnp", bufs=2))
        rcp = top.enter_context(tc.tile_pool(name="rcp", bufs=2))
        osc = top.enter_context(tc.tile_pool(name="osc", bufs=2))
        yev = top.enter_context(tc.tile_pool(name="yev", bufs=3))
        pps = top.enter_context(tc.tile_pool(name="pps", bufs=2, space="PSUM"))
        spool = top.enter_context(tc.tile_pool(name="sS", bufs=2, space="PSUM"))
        opool = top.enter_context(tc.tile_pool(name="sO", bufs=2, space="PSUM"))

        # persistent projection results
        kT_t = [per.tile([128, S], bf16, tag=f"kT{i}", name=f"kT{i}") for i in range(4)]
        v_st = [
            per.tile([128, 8 * 65], bf16, tag=f"v{i}", name=f"v{i}") for i in range(16)
        ]
        qq_t = [
            [per.tile([128, 512], bf16, tag=f"qq{i}_{j}", name=f"qq{i}_{j}")
             for j in range(4)]
            for i in range(4)
        ]
        AT_q = [
            [per.tile([128, 512], bf16, tag=f"AT{i}_{j}", name=f"AT{i}_{j}")
             for j in range(4)]
            for i in range(4)
        ]

        bq_sb = cst.tile([128, 4], f32, tag="bq")
        bk_sb = cst.tile([128, 4], f32, tag="bk")
        cexp = cst.tile([128, 1], f32, tag="cexp")
        nc.vector.memset(cexp[:], EXP_B)
        ones_bc = cst.tile([128, 64], bf16, tag="ones_bc")
        nc.vector.memset(ones_bc[:], 1.0)
        Wk_sb = wq.tile([128, 8, G_HEADS], bf16, tag="Wk")
        Wq_sb = wq.tile([128, 8, G_HEADS], bf16, tag="Wq")
        Wv_sb = wq.tile([128, 8, G_HEADS], bf16, tag="Wv")
        Wo_sb = wq.tile([128, 4, D], bf16, tag="Wo")

        # warm the exp table set early (one-time ~2.7us load)
        warm = cst.tile([128, 8], f32, tag="warm")
        nc.vector.memset(warm[:], 0.0)
        nc.scalar.activation(warm[:], warm[:], AF.Exp)

        # ---- DMA issues -----------------------------------------------------
        # per-ring DMA bandwidth is ~110GB/s, so spread the ~11MB of input
        # across all three rings (sync, scalar-hwdge, gpsimd-swdge) in
        # consumption order. The scalar queue gets ONLY first-use buffers:
        # a buffer-reusing issue would park a WAR wait in front of every
        # queued exp. vx2/vx3 (which reuse vx0/vx1) go on sync, whose later
        # work (AT shifts) is far away.
        qx_t = {}

        def dma_qx(qp, eng):
            qx_t[qp] = xqp.tile([128, 8, 512], bf16, tag="qx", name=f"qx{qp}")
            eng.dma_start(qx_t[qp][:], qsrc[:, qp])

        # All DMA rings share ~360GB/s of HBM and round-robin among queued
        # transfers, so everything issued up-front steals bandwidth from the
        # first-needed chunk. Issue only the phase-A-critical 3MB now; later
        # chunks are issued from the scalar queue BETWEEN pair-0 exps (the
        # exps act as a clock), or ride buffer-reuse WAR waits on sync.
        nc.gpsimd.dma_start(bq_sb[:], bqp[:, :])
        nc.gpsimd.dma_start(bk_sb[:], bkp[:, :])
        nc.scalar.dma_start(Wk_sb[:], WkT.ap().rearrange("p (d c) -> p d c", d=8))
        nc.scalar.dma_start(Wq_sb[:], WqT.ap().rearrange("p (d c) -> p d c", d=8))
        dma_qx(0, nc.scalar)
        nc.gpsimd.dma_start(Wv_sb[:], WvT.ap().rearrange("p (d c) -> p d c", d=8))
        nc.gpsimd.dma_start(Wo_sb[:], WoT.ap().rearrange("p (g n) -> p g n", g=4))

        with tc.tile_pool(name="kx", bufs=4) as kxp, \
             tc.tile_pool(name="vx", bufs=2) as vxp:
            kx = [kxp.tile([128, 8, 512], bf16, tag="kx", name=f"kx{sc}")
                  for sc in range(4)]
            vx = {}
            for sc in range(4):
                vx[sc] = vxp.tile([128, 8, 512], bf16, tag="vx", name=f"vx{sc}")
            nc.sync.dma_start(kx[0][:], ksrc[:, 0])
            # deferred issues, hooked onto pair-0 scalar slots (fresh buffers
            # only -- a WAR wait would block every queued exp behind it)
            DMAHOOK = {
                0: [lambda: nc.scalar.dma_start(vx[0][:], vsrc[:, 0])],
                1: [lambda: nc.scalar.dma_start(kx[1][:], ksrc[:, 1])],
                2: [lambda: nc.scalar.dma_start(vx[1][:], vsrc[:, 1])],
                3: [lambda: nc.scalar.dma_start(kx[2][:], ksrc[:, 2])],
                4: [lambda: nc.scalar.dma_start(kx[3][:], ksrc[:, 3])],
                # vx2/vx3 reuse vx0/vx1: WAR-gated, parked on sync
                5: [lambda: nc.sync.dma_start(vx[2][:], vsrc[:, 2]),
                    lambda: nc.sync.dma_start(vx[3][:], vsrc[:, 3])],
            }

            # ---- filler emitters -------------------------------------------
            def emit_kproj(sc, hp):
                ps = pps.tile([128, 512], f32, tag="ps", name=f"psk{sc}_{hp}")
                for dc in range(8):
                    nc.tensor.matmul(
                        ps[:],
                        Wk_sb[:, dc, hp * 128:(hp + 1) * 128],
                        kx[sc][:, dc, :],
                        start=(dc == 0),
                        stop=(dc == 7),
                    )
                nc.vector.tensor_scalar_add(
                    kT_t[hp][:, sc * 512:(sc + 1) * 512], ps[:], bk_sb[:, hp:hp + 1]
                )

            def emit_qproj(qp, hp):
                ps = pps.tile([128, 512], f32, tag="ps", name=f"psq{qp}_{hp}")
                for dc in range(8):
                    nc.tensor.matmul(
                        ps[:],
                        Wq_sb[:, dc, hp * 128:(hp + 1) * 128],
                        qx_t[qp][:, dc, :],
                        start=(dc == 0),
                        stop=(dc == 7),
                    )
                nc.vector.tensor_scalar_add(
                    qq_t[hp][qp][:], ps[:], bq_sb[:, hp:hp + 1]
                )

            def emit_vproj(st):
                sc, j = st // 4, st % 4
                ps = pps.tile([128, 512], f32, tag="ps", name=f"psv{st}")
                for dc in range(8):
                    nc.tensor.matmul(
                        ps[:],
                        vx[sc][:, dc, j * 128:(j + 1) * 128],
                        Wv_sb[:, dc, :],
                        start=(dc == 0),
                        stop=(dc == 7),
                    )
                vd = v_st[st][:].rearrange("p (h c) -> p h c", c=65)
                nc.vector.tensor_copy(
                    vd[:, :, 0:64], ps[:].rearrange("p (h c) -> p h c", c=64)
                )
                nc.vector.memset(vd[:, :, 64:65], 1.0)

            def emit_oproj(qp, chain):
                q4, nh = chain // 2, chain % 2
                qt_g = qp * 4 + q4
                ps = pps.tile([128, 512], f32, tag="ps", name=f"pso{qp}_{chain}")
                for hp in range(4):
                    nc.tensor.matmul(
                        ps[:],
                        AT_q[hp][qp][:, q4 * 128:(q4 + 1) * 128],
                        Wo_sb[:, hp, nh * 512:(nh + 1) * 512],
                        start=(hp == 0),
                        stop=(hp == 3),
                    )
                ye = yev.tile([128, 512], bf16, tag="ye")
                nc.vector.tensor_copy(ye[:], ps[:])
                (nc.gpsimd if chain % 2 else nc.sync).dma_start(
                    Y[qt_g * 128:(qt_g + 1) * 128, nh * 512:(nh + 1) * 512], ye[:]
                )

            # normalize pair (qp, hp): evict O + denominator row (bf16),
            # broadcast the denominator to 64 partitions with a K=1 matmul
            # (ones lhsT at row 64 -- lhsT/rhs must share base_partition),
            # reciprocal_approx_fast, multiply; hloc=1 partition-shifts into
            # AT[64:128] via SBUF->SBUF DMA. Spread over 4 steps (next pair's
            # slots 1-4) so the tensor-side bcast never waits on the dn copy.
            def emit_norm_step(qp, hp, O_t, st, step):
                if step == 0 or step == 1:
                    hloc = step
                    O = O_t[hloc]
                    dn = dnp.tile([128, 512], bf16, tag="dn")
                    nc.vector.tensor_copy(dn[64:65, :], O[64:65, :])
                    ov = oev.tile([128, 512], f32, tag="ov")
                    nc.vector.tensor_copy(ov[0:64, :], O[0:64, :])
                    st[hloc] = (dn, ov)
                    if step == 0:
                        return
                    # step 1 also kicks hloc0's broadcast + reciprocal
                    hloc = 0
                else:
                    hloc = 1
                dn, ov = st[hloc]
                denb = pps.tile([128, 512], f32, tag="ps", name=f"dnb{qp}{hp}{hloc}")
                nc.tensor.matmul(
                    denb[0:64, :], ones_bc[64:65, :], dn[64:65, :],
                    start=True, stop=True,
                )
                rcb = rcp.tile([128, 512], f32, tag="rcb")
                nc.vector.reciprocal_approx_fast(rcb[0:64, :], denb[0:64, :])
                st[2 + hloc] = rcb

            def emit_norm_mul(qp, hp, st, hloc):
                dn, ov = st[hloc]
                rcb = st[2 + hloc]
                if hloc == 0:
                    nc.vector.tensor_tensor(
                        AT_q[hp][qp][0:64, :], ov[0:64, :], rcb[0:64, :], MUL
                    )
                else:
                    sct = osc.tile([128, 512], bf16, tag="osc")
                    nc.vector.tensor_tensor(
                        sct[0:64, :], ov[0:64, :], rcb[0:64, :], MUL
                    )
                    nc.sync.dma_start(AT_q[hp][qp][64:128, :], sct[0:64, :])

            def emit_norm(qp, hp, O_t):
                st = [None, None, None, None]
                emit_norm_step(qp, hp, O_t, st, 0)
                emit_norm_step(qp, hp, O_t, st, 1)
                emit_norm_step(qp, hp, O_t, st, 2)
                emit_norm_mul(qp, hp, st, 0)
                emit_norm_mul(qp, hp, st, 1)

            # ---- attention pair --------------------------------------------
            # Boundary discipline: the PREVIOUS pair's last 3 PVs and one
            # filler chain are emitted BETWEEN this pair's first scores, so
            # the tensor queue has work while exp(0)/exp(1) free the 2-deep
            # S-PSUM rotation (without it, every pair start stalls ~2us).
            def emit_pair(i, fillers, boundary, prev_tail, pre, post=(),
                          dmahook=None):
                qp, hp = i // 4, i % 4
                kt = kT_t[hp]
                qtile = qq_t[hp][qp]
                O_t = [
                    opool.tile([128, 512], f32, tag="O", name=f"O{i}_0"),
                    opool.tile([128, 512], f32, tag="O", name=f"O{i}_1"),
                ]
                P_of = {}

                def emit_pv(kc):
                    Pt = P_of[kc]
                    for hloc in range(2):
                        lv = v_st[kc][:, (2 * hp + hloc) * 65:(2 * hp + hloc) * 65 + 65]
                        nc.tensor.matmul(
                            O_t[hloc][0:65, :],
                            lv,
                            Pt[:, hloc * 512:(hloc + 1) * 512],
                            start=(kc == 0),
                            stop=(kc == 15),
                        )

                for j in range(16):
                    S_big = spool.tile([128, 1024], f32, tag="S", name=f"S{i}_{j}")
                    for hloc in range(2):
                        nc.tensor.matmul(
                            S_big[:, hloc * 512:(hloc + 1) * 512],
                            kt[hloc * 64:hloc * 64 + 64, j * 128:(j + 1) * 128],
                            qtile[hloc * 64:hloc * 64 + 64, :],
                            start=True,
                            stop=True,
                        )
                    Pt = ppool.tile([128, 1024], bf16, tag="P", name="P")
                    nc.scalar.activation(Pt[:], S_big[:], AF.Exp, scale=0.125)
                    P_of[j] = Pt
                    if dmahook is not None and j in dmahook:
                        for fn in dmahook[j]:
                            fn()
                    if j == 0 and prev_tail:
                        prev_tail[0]()
                    elif j == 1 and prev_tail:
                        prev_tail[1]()
                        prev_tail[2]()
                    if pre is not None and j in pre:
                        for fn in pre[j]:
                            fn()
                    if j == 3:
                        for fn in boundary:
                            fn()
                    if j in fillers:
                        for fn in fillers[j]:
                            fn()
                    if j >= 3:
                        emit_pv(j - 3)
                for fn in post:
                    fn()
                return O_t, [lambda: emit_pv(13), lambda: emit_pv(14),
                             lambda: emit_pv(15)]

            # ---- phase A: first K / Q projections --------------------------
            emit_kproj(0, 0)
            emit_qproj(0, 0)

            # ---- pair schedule ---------------------------------------------
            # FILL[i]: slot -> [filler closures]; BND[i]: boundary fillers
            K, Q, V, Og = emit_kproj, emit_qproj, emit_vproj, emit_oproj
            norm_t = {}  # pair i -> (qp, hp, O_t) awaiting normalize

            def mknorm(i):
                qp0, hp0 = i // 4, i % 4
                st = [None, None, None, None]

                def _step(k):
                    def _n():
                        O0 = norm_t[i][2]
                        if k < 3:
                            emit_norm_step(qp0, hp0, O0, st, k)
                            if k == 2:
                                emit_norm_mul(qp0, hp0, st, 0)
                        else:
                            emit_norm_mul(qp0, hp0, st, 1)
                    return _n
                return {1: [_step(0)], 2: [_step(1)], 3: [_step(2)],
                        4: [_step(3)]}

            FILL = {
                0: {3: [lambda: K(1, 0)], 6: [lambda: K(2, 0)],
                    9: [lambda: K(3, 0)], 10: [lambda: K(0, 1)],
                    11: [lambda: K(1, 1)], 12: [lambda: K(2, 1)],
                    13: [lambda: K(3, 1)], 14: [lambda: Q(0, 1)]},
                1: {1: [lambda: K(1, 2)], 3: [lambda: K(2, 2)],
                    5: [lambda: K(3, 2)], 7: [lambda: K(0, 3)],
                    9: [lambda: K(1, 3)], 11: [lambda: K(2, 3)],
                    13: [lambda: K(3, 3)], 6: [lambda: Q(0, 2)]},
                2: {},
                3: {8: [lambda: Q(1, 1)]},
                4: {6: [lambda: Og(0, 0)], 10: [lambda: Og(0, 1)],
                    13: [lambda: Q(1, 3)]},
                5: {6: [lambda: Q(2, 0)], 10: [lambda: Og(0, 3)]},
                6: {6: [lambda: Q(2, 1)], 10: [lambda: Og(0, 5)]},
                7: {6: [lambda: Q(2, 2)], 10: [lambda: Og(0, 7)]},
                8: {6: [lambda: Og(1, 0)], 10: [lambda: Og(1, 1)]},
                9: {6: [lambda: Q(3, 0)], 10: [lambda: Og(1, 3)]},
                10: {6: [lambda: Q(3, 1)], 10: [lambda: Og(1, 5)]},
                11: {6: [lambda: Q(3, 2)], 10: [lambda: Og(1, 7)]},
                12: {6: [lambda: Og(2, 0)], 10: [lambda: Og(2, 1)]},
                13: {8: [lambda: Og(2, 3)]},
                14: {8: [lambda: Og(2, 5)]},
                15: {8: [lambda: Og(2, 7)]},
            }
            BND = {
                1: [lambda: K(0, 2)],
                2: [lambda: Q(0, 3)],
                3: [lambda: Q(1, 0)],
                4: [lambda: Q(1, 2)],
                5: [lambda: Og(0, 2)],
                6: [lambda: Og(0, 4)],
                7: [lambda: Og(0, 6)],
                8: [lambda: Q(2, 3)],
                9: [lambda: Og(1, 2)],
                10: [lambda: Og(1, 4)],
                11: [lambda: Og(1, 6)],
                12: [lambda: Q(3, 3)],
                13: [lambda: Og(2, 2)],
                14: [lambda: Og(2, 4)],
                15: [lambda: Og(2, 6)],
            }

            prev_tail = None
            for i in range(16):
                qp, hp = i // 4, i % 4
                fillers = dict(FILL[i])
                post = ()
                if i == 0:
                    # V projections ride pair-0 slots: V(0),V(1) at slot 3
                    # (vx0 lands ~25us), V(st) at slot st+2 after; V(14)/
                    # V(15) land right after slot 15, before the tail PVs
                    # fire at the pair-1 boundary
                    fillers.setdefault(3, [])
                    fillers[3] = [lambda: V(0), lambda: V(1)] + fillers[3]
                    for st in range(2, 14):
                        fillers.setdefault(st + 2, [])
                        fillers[st + 2] = [lambda s=st: V(s)] + fillers[st + 2]
                    post = (lambda: V(14), lambda: V(15))
                # prev pair's normalize lands in slots 1-4 of this pair
                pre = mknorm(i - 1) if i > 0 else None
                # qx prefetches on the gpsimd queue
                if i == 1:
                    dma_qx(1, nc.gpsimd)
                elif i == 5:
                    dma_qx(2, nc.gpsimd)
                elif i == 7:
                    dma_qx(3, nc.gpsimd)
                O_t, tail = emit_pair(
                    i, fillers, BND.get(i, []), prev_tail, pre, post=post,
                    dmahook=DMAHOOK if i == 0 else None,
                )
                norm_t[i] = (qp, hp, O_t)
                prev_tail = tail

        # ---- tail: last pair's PVs + normalize + final out-proj chains -----
        for fn in prev_tail:
            fn()
        emit_norm(3, 3, norm_t[15][2])
        for chain in range(8):
            emit_oproj(3, chain)

    nc.compile()
    return nc


_NC = None


def _get_nc():
    global _NC
    if _NC is None:
        _NC = build()
    return _NC


def _wpack(WT):
    # [D, C] -> [128, 8*C] with row p holding [WT[dc*128+p, :] for dc in 0..8]
    D_, C = WT.shape
    d = D_ // 128
    return WT.reshape(d, 128, C).transpose(1, 0, 2).reshape(128, d * C)


def _xpack(Xb, bf):
    # [S, D] -> [128, 4*8*512]: row p holds [X.T[d*128+p, sc*512:(sc+1)*512]
    # for sc in 0..4 for d in 0..8] (chunk-major, contiguous 8KB DMA slices)
    XT = Xb.T  # [D, S]
    Hm = XT.reshape(8, 128, 4, 512)  # [d, p, sc, s']
    return np.ascontiguousarray(
        Hm.transpose(1, 2, 0, 3).astype(bf).reshape(128, 4 * 8 * 512)
    )


def _prep_core(Q, K, V, Wq, bq, Wk, bk, Wv, Wo, b, g):
    c = np.ascontiguousarray
    bf = ml_dtypes.bfloat16
    hs = slice(g * G_HEADS, (g + 1) * G_HEADS)
    return {
        "Qp": _xpack(Q[b], bf),
        "Kp": _xpack(K[b], bf),
        "Vp": _xpack(V[b], bf),
        "WqT": c(_wpack(Wq[hs, :].T.astype(bf))),
        "WkT": c(_wpack(Wk[hs, :].T.astype(bf))),
        "WvT": c(_wpack(Wv[hs, :].T.astype(bf))),
        "WoT": c(_wpack(Wo[:, hs].T.astype(bf))),
        "bqp": c(bq[hs].reshape(4, 128).T),
        "bkp": c(bk[hs].reshape(4, 128).T),
    }


def kernel(Q, K, V, Wq, bq, Wk, bk, Wv, bv, Wo, bo, _want_trace=False):
    Q, K, V = (np.asarray(x, np.float32) for x in (Q, K, V))
    Wq, bq, Wk, bk, Wv, bv, Wo, bo = (
        np.asarray(x, np.float32) for x in (Wq, bq, Wk, bk, Wv, bv, Wo, bo)
    )
    nc = _get_nc()
    in_maps = [
        _prep_core(Q, K, V, Wq, bq, Wk, bk, Wv, Wo, b=c % 4, g=c // 4)
        for c in range(NCORES)
    ]
    res = run_bass_kernel_spmd(
        nc, in_maps, core_ids=list(range(NCORES)), trace=_want_trace
    )
    out = np.zeros((B, S, D), np.float32)
    for c in range(NCORES):
        out[c % 4] += res.results[c]["Y"].astype(np.float32)
    out += (bo + Wo.astype(np.float64) @ bv.astype(np.float64)).astype(np.float32)[
        None, None, :
    ]
    if _want_trace:
        kernel.last_exec_time_ns = res.exec_time_ns
        kernel.last_trace = res.instructions_and_trace
    return out
